# revision 1
# baseline (speedup 1.0000x reference)
"""MoE routing gate (nn_Gate) for 8 trn2 NeuronCores.

Contract: kernel(x, weight, bias) takes the FULL inputs and returns
(weights[16384,2] f32, indices[16384,2] i32, aux_loss f32 scalar),
matching reference.reference().

Strategy: data-parallel over tokens (8 cores x 2048 tokens). Router weight
replicated. Host pre-packs x as transposed bf16 hi/lo pairs (exact fp32
split: x == hi + lo to within 2^-17 rel) so the PE runs 1 cycle/row bf16
matmuls; aux-loss statistics are tiny per-core partials finished on host.
"""

import sys

sys.path.insert(0, "/opt/trn_rl_repo")
sys.path.insert(0, "/root/problem")

import numpy as np

T_TOTAL = 16384
D = 2048
E = 8
TOPK = 2
N_CORES = 8
T_CORE = T_TOTAL // N_CORES
ALPHA = 1e-4

_CACHE = {}


def _get_program():
    if "nc" not in _CACHE:
        from gate_kernel import build_gate_program

        _CACHE["nc"] = build_gate_program()
    return _CACHE["nc"]


def last_exec_time_ns():
    return _CACHE.get("exec_time_ns")


def kernel(x, weight, bias, _trace=False):
    from gate_kernel import host_inputs_for_core, host_shared_inputs
    from concourse.bass_utils import run_bass_kernel_spmd

    x = np.ascontiguousarray(np.asarray(x, dtype=np.float32))
    weight = np.asarray(weight, dtype=np.float32)
    bias = np.asarray(bias, dtype=np.float32)

    nc = _get_program()
    wp, bias_rep, iota_rep, iota8_rep, id8 = host_shared_inputs(weight, bias)
    in_maps = [
        host_inputs_for_core(
            x[c * T_CORE:(c + 1) * T_CORE], wp, bias_rep, iota_rep, iota8_rep, id8
        )
        for c in range(N_CORES)
    ]
    res = run_bass_kernel_spmd(
        nc, in_maps, list(range(N_CORES)), trace=_trace
    )
    _CACHE["exec_time_ns"] = res.exec_time_ns
    _CACHE["results"] = res

    w_full = np.concatenate([res.results[c]["wout"] for c in range(N_CORES)], axis=0)
    i_full = np.concatenate([res.results[c]["iout"] for c in range(N_CORES)], axis=0)

    # aux loss: ce from the (exact) integer counts, Pi from device prob-sums
    counts = np.bincount(i_full.reshape(-1), minlength=E).astype(np.float64)
    ce = counts / (T_TOTAL * TOPK)
    pi_sum = np.zeros(E, dtype=np.float64)
    for c in range(N_CORES):
        pi_sum += res.results[c]["psums"].astype(np.float64).sum(axis=(0, 1))
    Pi = pi_sum / T_TOTAL
    aux = np.float32((Pi * ce * E).sum() * ALPHA)

    return w_full.astype(np.float32), i_full.astype(np.int32), aux
